# revision 1
# baseline (speedup 1.0000x reference)
"""BERT parallel self-attention on 8 Trainium2 NeuronCores (Bass/Tile).

Self-contained: kernel(**inputs) takes the FULL inputs
  hidden_states [2, 4096, 768] f32, attention_mask [2, 1, 1, 4096] f32,
  W_qkv [768, 2304] f32, b_qkv [2304] f32
and returns the FULL context output [2, 4096, 768] f32.

Sharding (Megatron-style tensor-parallel over heads + data-parallel over
batch): core c handles batch c//4, heads 3*(c%4)..3*(c%4)+2. Each core runs
an identical SPMD program on its shard; host gathers the 8 outputs.

Per-core device program:
  1. hidden -> bf16 -> DMA-xbar transpose -> hT [768, S] (h on partitions)
  2. mixed_T[f, t] = W^T hsT (PE, bf16). Host packs W columns
     [Q0|Q1|K0|K1|Q2|K2|V0|V1|V2] so f-block 0 -> Q_T of heads 0,1 stacked at
     partitions 0-63/64-127 (row-pair layout for the 128x128 PE array),
     f-block 1 -> K_T likewise, f-block 2 -> head 2 (duplicated to both
     halves). V is computed in natural [t, f] orientation with an appended
     ones column (softmax denominator rides the ctx matmul).
  3. attention per (q-chunk, t-block): scores_T[t, q] via two row-packed
     K=64 matmuls; exp on ScalarE reading 2 PSUM banks per instruction
     (scale=1/8 folds the 1/sqrt(sqrt(hn))^2 norm, bias=mask[t] per
     partition); ctx_T[65, q] += [V|1]^T expS accumulated in PSUM over t.
  4. per (head, q-chunk): PE-transpose 128-blocks of [ctx_T|Z] -> [q, 65],
     reciprocal of Z column, scale, store [q, 192] f32.
"""

from contextlib import ExitStack

import ml_dtypes
import numpy as np

import concourse.bass as bass
import concourse.mybir as mybir
import concourse.tile as tile
from concourse import bacc
from concourse.bass import ts
from concourse.bass_utils import run_bass_kernel_spmd
from concourse.masks import make_identity

F32 = mybir.dt.float32
BF16 = mybir.dt.bfloat16
EXP = mybir.ActivationFunctionType.Exp

P = 128
HH = 768          # hidden size
HB = HH // P      # 6 h-blocks
NHEAD = 3         # heads per core
HN = 64
FQKV = 576        # packed feature columns per core
QCHUNK = 512
B, S, H = 2, 4096, 768
N_CORES = 8


def _build(nc: bass.Bass, S: int = S):
    TB = S // P               # token blocks
    QC = S // QCHUNK          # q chunks
    assert QC % 2 == 0

    hs_d = nc.dram_tensor("hs", [S, HH], BF16, kind="ExternalInput").ap()
    w_d = nc.dram_tensor("w", [HH, FQKV], F32, kind="ExternalInput").ap()
    b_d = nc.dram_tensor("b", [640, 1], F32, kind="ExternalInput").ap()
    bflat_d = nc.dram_tensor("bflat", [1, 640], F32, kind="ExternalInput").ap()
    mask_d = nc.dram_tensor("mask", [S, 1], F32, kind="ExternalInput").ap()
    out_d = nc.dram_tensor("out", [S, NHEAD * HN], F32, kind="ExternalOutput").ap()

    with tile.TileContext(nc) as tc, ExitStack() as st_p:
        pool_p = st_p.enter_context(tc.tile_pool(name="persist", bufs=1))

        hT = pool_p.tile([P, HB, S], BF16, tag="hT")
        QT01 = pool_p.tile([P, S], BF16, tag="QT01")
        KT01 = pool_p.tile([P, S], BF16, tag="KT01")
        QT2 = pool_p.tile([P, S], BF16, tag="QT2")
        KT2 = pool_p.tile([P, S], BF16, tag="KT2")
        VZ = pool_p.tile([P, TB, NHEAD, HN + 1], BF16, tag="VZ")
        outsb = pool_p.tile([P, TB, NHEAD * HN], F32, tag="outsb")
        wb = pool_p.tile([P, HB, FQKV], BF16, tag="wb")
        btile = pool_p.tile([P, 5], F32, tag="btile")
        bvrow = pool_p.tile([1, NHEAD * HN], F32, tag="bvrow")
        bvb = pool_p.tile([P, NHEAD, HN], F32, tag="bvb")
        masks = pool_p.tile([P, TB], F32, tag="masks")
        ident = pool_p.tile([P, P], F32, tag="ident")
        identb = pool_p.tile([P, P], BF16, tag="identb")

        make_identity(nc, ident[:])
        nc.vector.tensor_copy(identb[:], ident[:])
        nc.vector.memset(VZ[:, :, :, HN : HN + 1], 1.0)


        # ---- phase 1+2: load/cast/transpose hidden; QKV projection ----
        with ExitStack() as st_12:
            pool_ld = st_12.enter_context(tc.tile_pool(name="ld", bufs=3))
            pool_qkps = st_12.enter_context(
                tc.tile_pool(name="qkps", bufs=3, space="PSUM")
            )
            pool_vps = st_12.enter_context(
                tc.tile_pool(name="vps", bufs=2, space="PSUM")
            )
            pool_bv = pool_vps

            wf32 = pool_ld.tile([P, HB, FQKV], F32, tag="wf32")
            for hb in range(HB):
                nc.scalar.dma_start(out=wf32[:, hb, :], in_=w_d[ts(hb, P), :])
            nc.vector.tensor_copy(wb[:], wf32[:])

            # small loads on the scalar HWDGE queue (sync queue is for hidden)
            for fb in range(5):
                nc.scalar.dma_start(out=btile[:, fb : fb + 1], in_=b_d[ts(fb, P), :])
            nc.scalar.dma_start(out=bvrow[:], in_=bflat_d[:, 384:576])
            for tb in range(TB):
                nc.scalar.dma_start(
                    out=masks[:, tb : tb + 1], in_=mask_d[ts(tb, P), :]
                )

            # V-bias broadcast row -> [128, 192] via K=1 matmul
            ones1 = pool_ld.tile([1, P], F32, tag="ones1")
            nc.vector.memset(ones1[:], 1.0)
            bvps = pool_bv.tile([P, NHEAD, HN], F32, tag="bvps")
            nc.tensor.matmul(bvps[:], ones1[:], bvrow[:], start=True, stop=True)
            nc.vector.tensor_copy(bvb[:], bvps[:])

            # hidden -> hT via DMA-xbar transpose straight from DRAM (bf16),
            # in token-halves so early QKV/V matmuls can start sooner.
            # All transposes stay on ONE queue: concurrent xbar transposes on
            # two HWDGE queues corrupt data (shared-xbar hazard, measured).
            SH = S // 2
            for half in range(2):
                for hb in range(HB):
                    nc.sync.dma_start_transpose(
                        out=hT[:, hb, ts(half, SH)],
                        in_=hs_d[ts(half, SH), ts(hb, P)],
                    )

            TPQ = QCHUNK // P  # token blocks per chunk
            for tq in range(S // QCHUNK):
                for tbl in range(TPQ):
                    tb = tq * TPQ + tbl
                    # V natural orientation: lhsT = hT blocks, rhs = W_v cols
                    vv = pool_vps.tile([P, NHEAD, HN], F32, tag="vv")
                    for hb in range(HB):
                        nc.tensor.matmul(
                            vv[:],
                            hT[:, hb, ts(tb, P)],
                            wb[:, hb, 384:576],
                            start=(hb == 0),
                            stop=(hb == HB - 1),
                        )
                    nc.vector.tensor_tensor(
                        VZ[:, tb, :, 0:HN], vv[:], bvb[:], op=mybir.AluOpType.add
                    )

                # mixed_T f-blocks (Q0Q1, K0K1, Q2K2) for this token chunk
                for fb in range(3):
                    mm = pool_qkps.tile([P, QCHUNK], F32, tag="mm")
                    for hb in range(HB):
                        nc.tensor.matmul(
                            mm[:],
                            wb[:, hb, ts(fb, P)],
                            hT[:, hb, ts(tq, QCHUNK)],
                            start=(hb == 0),
                            stop=(hb == HB - 1),
                        )
                    dst = ts(tq, QCHUNK)
                    if fb == 0:
                        nc.vector.tensor_scalar_add(
                            QT01[:, dst], mm[:], btile[:, 0:1]
                        )
                    elif fb == 1:
                        nc.vector.tensor_scalar_add(
                            KT01[:, dst], mm[:], btile[:, 1:2]
                        )
                    else:
                        nc.vector.tensor_scalar_add(
                            QT2[0:HN, dst], mm[0:HN, :], btile[0:HN, 2:3]
                        )
                        nc.vector.tensor_scalar_add(
                            KT2[HN:P, dst], mm[HN:P, :], btile[HN:P, 2:3]
                        )
            # duplicate head-2 Q/K to the other partition half
            nc.sync.dma_start(out=QT2[HN:P, :], in_=QT2[0:HN, :])
            nc.sync.dma_start(out=KT2[0:HN, :], in_=KT2[HN:P, :])

        # ---- phase 3: attention ----
        with ExitStack() as st_3:
            pool_sc = st_3.enter_context(tc.tile_pool(name="sc", bufs=2, space="PSUM"))
            pool_ct = st_3.enter_context(tc.tile_pool(name="ct", bufs=2, space="PSUM"))
            pool_tp = pool_ct  # transpose psum rides the just-freed ct slot
            pool_es = st_3.enter_context(tc.tile_pool(name="es", bufs=3))
            pool_cts = st_3.enter_context(tc.tile_pool(name="cts", bufs=2))
            pool_rz = st_3.enter_context(tc.tile_pool(name="rz", bufs=2))

            def postprocess(ct, head, qc):
                """ct: psum [65, 512] = [ctx_T ; Z] -> normalized out[q, 64]."""
                cts = pool_cts.tile([HN + 1, QCHUNK], F32, tag="cts")
                nc.vector.tensor_copy(cts[:], ct[:])
                for j in range(QCHUNK // P):
                    tp = pool_tp.tile([P, HN + 1], F32, tag="ctA")
                    nc.tensor.transpose(
                        tp[:], cts[:, ts(j, P)], ident[0 : HN + 1, 0 : HN + 1]
                    )
                    rz = pool_rz.tile([P, 1], F32, tag="rz")
                    nc.vector.reciprocal(rz[:], tp[:, HN : HN + 1])
                    tb_out = qc * (QCHUNK // P) + j
                    nc.vector.tensor_scalar_mul(
                        outsb[:, tb_out, ts(head, HN)], tp[:, 0:HN], rz[:]
                    )

            def attn_step(tb, kA, qA, kB, qB, ctA, ctB, vA, vB, first, last):
                """One t-block: row-packed scores pair, exp, two ctx matmuls."""
                sc = pool_sc.tile([P, 2, QCHUNK], F32, tag="sc")
                nc.tensor.matmul(sc[:, 0, :], kA, qA, start=True, stop=True)
                nc.tensor.matmul(sc[:, 1, :], kB, qB, start=True, stop=True)
                es = pool_es.tile([P, 2, QCHUNK], BF16, tag="es")
                nc.scalar.activation(
                    es[:], sc[:], EXP, bias=masks[:, tb : tb + 1], scale=0.125
                )
                nc.tensor.matmul(
                    ctA[:], vA, es[:, 0, :], start=first, stop=last,
                    skip_group_check=True,
                )
                nc.tensor.matmul(
                    ctB[:], vB, es[:, 1, :], start=first, stop=last,
                    skip_group_check=True,
                )

            # (pair-half A args, pair-half B args) per phase step; postprocess
            # of the previous step is deferred past the first few attn_steps
            # of the next so the PE queue never stalls ACT at qc boundaries.
            steps = []
            for qc in range(QC):  # heads 0,1 (partition-paired)
                steps.append((QT01, KT01, (0, qc), (1, qc), 0, 1))
            for qcp in range(QC // 2):  # head 2 (self-paired across q-chunks)
                steps.append((QT2, KT2, (2, 2 * qcp), (2, 2 * qcp + 1), 2, 2))

            pending = None
            for QT, KT, (hA, qcA), (hB, qcB), hvA, hvB in steps:
                ctA = pool_ct.tile([HN + 1, QCHUNK], F32, tag="ctA")
                ctB = pool_ct.tile([HN + 1, QCHUNK], F32, tag="ctB")
                for tb in range(TB):
                    attn_step(
                        tb,
                        KT[0:HN, ts(tb, P)], QT[0:HN, ts(qcA, QCHUNK)],
                        KT[HN:P, ts(tb, P)], QT[HN:P, ts(qcB, QCHUNK)],
                        ctA, ctB,
                        VZ[:, tb, hvA, :], VZ[:, tb, hvB, :],
                        tb == 0, tb == TB - 1,
                    )
                    if tb == 2 and pending is not None:
                        for ct, h, qc in pending:
                            postprocess(ct, h, qc)
                        pending = None
                pending = [(ctA, hA, qcA), (ctB, hB, qcB)]
            for ct, h, qc in pending:
                postprocess(ct, h, qc)

            for tb in range(TB):
                nc.sync.dma_start(out=out_d[ts(tb, P), :], in_=outsb[:, tb, :])


_NC_CACHE = None


def _get_nc():
    global _NC_CACHE
    if _NC_CACHE is None:
        nc = bacc.Bacc(
            "TRN2", target_bir_lowering=False, debug=False, num_devices=N_CORES
        )
        _build(nc)
        nc.compile()
        _NC_CACHE = nc
    return _NC_CACHE


def _shard_inputs(hidden_states, attention_mask, W_qkv, b_qkv):
    in_maps = []
    for c in range(N_CORES):
        b, hg = c // 4, c % 4
        h0 = 3 * hg
        order = [(0, h0), (0, h0 + 1), (768, h0), (768, h0 + 1),
                 (0, h0 + 2), (768, h0 + 2),
                 (1536, h0), (1536, h0 + 1), (1536, h0 + 2)]
        cols = np.concatenate(
            [np.arange(off + h * HN, off + (h + 1) * HN) for off, h in order]
        )
        w = np.ascontiguousarray(W_qkv[:, cols], dtype=np.float32)
        bv = np.zeros(640, dtype=np.float32)
        bv[:FQKV] = b_qkv[cols]
        in_maps.append(
            {
                "hs": np.ascontiguousarray(hidden_states[b]).astype(
                    ml_dtypes.bfloat16
                ),
                "w": w,
                "b": bv[:, None].copy(),
                "bflat": bv[None, :].copy(),
                "mask": np.ascontiguousarray(
                    attention_mask[b, 0, 0, :, None], dtype=np.float32
                ),
            }
        )
    return in_maps


def _unshard(results):
    out = np.empty((B, S, H), dtype=np.float32)
    for c, r in enumerate(results):
        b, hg = c // 4, c % 4
        out[b, :, hg * 192 : (hg + 1) * 192] = r["out"]
    return out


def kernel(hidden_states, attention_mask, W_qkv, b_qkv, _trace=False, _tmpdir=None):
    nc = _get_nc()
    in_maps = _shard_inputs(
        np.asarray(hidden_states), np.asarray(attention_mask),
        np.asarray(W_qkv), np.asarray(b_qkv),
    )
    res = run_bass_kernel_spmd(
        nc, in_maps, core_ids=list(range(N_CORES)), trace=_trace, tmpdir=_tmpdir
    )
    out = _unshard(res.results)
    if _trace:
        kernel.last_exec_time_ns = res.exec_time_ns
        kernel.last_results = res
    return out

